# revision 20
# baseline (speedup 1.0000x reference)
"""Complex multi-head attention on 8 Trainium2 cores (Bass/Tile).

Sharding: pure data-parallel over batch (B=8 -> 1 batch per core),
weights replicated. No collectives.

Per-core dataflow (batch b):
  - Host supplies feature-major activations XT = [xr.T; xi.T] [1024, S]
    in bf16 and repacked/sign-folded weights (bf16) so every complex
    linear is one stacked-K real matmul chain.
  - V-projection (all heads) -> V1 token-major [t, (h, vr|vi)] (fp32r).
  - Per head h: Q/K projections -> feature-major stacks [(c,dh)=128, S]
    fp32r; scores computed TRANSPOSED (K-stationary): comp0 lhsT =
    kneg = [kr; -ki], comp1 lhsT = kswap = [ki; kr], rhs = qstack for
    BOTH comps; softmax without max-subtraction (|s| <= ~16);
    row sums via ones-matmuls (partition reduce + broadcast in one);
    reciprocal via the ~5x-faster approx-fast DVE op; normalization
    fused into P1/P2 PSUM evacuation.
  - Software pipelining: head h+1's K-projection block is emitted
    between attn(h, nh=0) and attn(h, nh=1), and its Q-projection
    block after attn(h, nh=1), so the tensor engine never waits on the
    DVE normalize chain or projection evacuations. Head 0's projection
    blocks are interleaved into the V-projection phase. Scores are
    emitted with one-tk lookahead so exp latency is off the PE
    critical path.
  - Output projection accumulates heads as K-chunks -> [t, (o, c)]
    (bf16 osb x bf16 wo), which is exactly the [S, D, 2] DRAM layout.
"""

import sys
import types
import numpy as np
import ml_dtypes

B, S, D, H = 8, 1024, 512, 8
DH = D // H
KC = 8  # k-chunks of 128 over (c,d) = 1024
TC = 8  # token chunks of 128
NCORES = 8

LAST_EXEC_NS = None


# ---------------------------------------------------------------- shims
def _install_axon_profile_shim():
    if "antenv.axon_hooks" in sys.modules:
        return
    try:
        import antenv  # noqa: F401

        mod = types.ModuleType("antenv.axon_hooks")
        state = {"hook": None}
        mod.set_axon_ntff_profile_hook = lambda h: state.__setitem__("hook", h)
        mod.get_axon_ntff_profile_hook = lambda: state["hook"]
        sys.modules["antenv.axon_hooks"] = mod
        from trn_agent_boot.trn_boot import _ntff_profile_via_ctypes

        hook = _ntff_profile_via_ctypes("/opt/axon/libaxon_pjrt.so")
        if hook is not None:
            mod.set_axon_ntff_profile_hook(hook)
    except Exception:
        pass


def _install_tile_drain_patch():
    """This walrus build allows ONE sync wait per instruction; split the
    TileContext exit drain's waits across preceding sync NOPs."""
    import concourse.mybir as mybir
    import concourse.tile as tile
    from concourse.vector_clock import ScopedClock

    if getattr(tile.TileContext, "_drain_patched", False):
        return

    def _patched(self, tick_clock, wait_clock):
        probe = mybir.InstNoOp(name="I-drain-probe")
        probe.engine = mybir.EngineType.SP
        wait_clock.add_sem_waits(probe, ScopedClock({None: tick_clock.global_clock}))
        waits = list(probe.sync_info.on_wait or []) if probe.sync_info else []
        for w in waits:
            nop = self.nc.sync.nop()
            nop.ins.sync_info = mybir.SyncInfo(on_wait=[w], on_update=[])
        self.nc.sync.drain()
        self.nc.all_engine_barrier()
        assert self.sems is not None
        popped = self.nc._tile_sem_poison_stack.pop()
        assert popped is self._sem_poison
        self.nc.clear_and_free_semaphores(list(self.sems.allocated().values()))
        self.nc.all_engine_barrier()

    tile.TileContext._drain_and_barrier = _patched
    tile.TileContext._drain_patched = True


def _split_waits(nc, max_waits=1):
    """Hoist extra sync waits onto preceding same-engine NOPs (walrus here
    rejects >1 sync wait per instruction)."""
    import concourse.mybir as mybir

    def process(blk):
        lst = blk.instructions
        i = 0
        while i < len(lst):
            inst = lst[i]
            if hasattr(inst, "blocks"):
                for b in inst.blocks or []:
                    process(b)
            si = inst.sync_info
            if si is not None and si.on_wait and len(si.on_wait) > max_waits:
                waits = list(si.on_wait)
                keep, extra = waits[-max_waits:], waits[:-max_waits]
                inst.sync_info = mybir.SyncInfo(
                    on_wait=keep, on_update=list(si.on_update or [])
                )
                for j, w in enumerate(extra):
                    nop = mybir.InstNoOp(name=f"{inst.name}-ws{j}")
                    nop.engine = inst.engine
                    nop.sync_info = mybir.SyncInfo(on_wait=[w], on_update=[])
                    lst.insert(i, nop)
                    i += 1
            i += 1

    for f in nc.m.functions:
        for blk in f.blocks:
            process(blk)


# ------------------------------------------------------------ host prep
def _build_wqk(wr, wi, scale):
    """[1024 k=(c,d), 1024 m=(h, c', dh)] for Q/K/V projections."""
    W = np.empty((2 * D, 2 * D), np.float32)
    for h in range(H):
        o = slice(h * DH, (h + 1) * DH)
        c0 = h * 2 * DH
        W[0:D, c0 : c0 + DH] = wr[o].T * scale
        W[D:, c0 : c0 + DH] = -wi[o].T * scale
        W[0:D, c0 + DH : c0 + 2 * DH] = wi[o].T * scale
        W[D:, c0 + DH : c0 + 2 * DH] = wr[o].T * scale
    return W


def _head_tiles(W):
    """[1024,1024] -> [H, 128, 1024]: per-head column block, k-chunk cols."""
    out = np.empty((H, 128, 1024), np.float32)
    for h in range(H):
        blk = W[:, h * 128 : (h + 1) * 128]  # [1024, 128]
        for kk in range(KC):
            out[h, :, kk * 128 : (kk + 1) * 128] = blk[kk * 128 : (kk + 1) * 128]
    return out


def _kchunk_tiles(W):
    """[1024,1024] -> [KC, 128, 1024]: row chunks."""
    return np.ascontiguousarray(W.reshape(KC, 128, 1024))


def _build_wo(wo_r, wo_i):
    """rows (h, c', dh), cols (o, c) interleaved to match [S, D, 2]."""
    W = np.empty((2 * D, 2 * D), np.float32)
    for h in range(H):
        d = slice(h * DH, (h + 1) * DH)
        r0 = h * 2 * DH
        W[r0 : r0 + DH, 0::2] = wo_r[:, d].T
        W[r0 : r0 + DH, 1::2] = wo_i[:, d].T
        W[r0 + DH : r0 + 2 * DH, 0::2] = -wo_i[:, d].T
        W[r0 + DH : r0 + 2 * DH, 1::2] = wo_r[:, d].T
    return W


def _xt(x):  # [S, D, 2] -> [2D, S] feature-major
    out = np.empty((2 * D, S), np.float32)
    out[0:D] = x[:, :, 0].T
    out[D:] = x[:, :, 1].T
    return out


def _bf16(a):
    return np.ascontiguousarray(a).astype(ml_dtypes.bfloat16)


# ------------------------------------------------------------ bass build
def _build_nc():
    import concourse.bass as bass
    import concourse.mybir as mybir
    import concourse.tile as tile
    from contextlib import ExitStack

    MDT = mybir.dt.float32r
    BF = mybir.dt.bfloat16
    F32 = mybir.dt.float32

    nc = bass.Bass()
    d_xtq = nc.dram_tensor("xtq", [KC, 128, S], BF, kind="ExternalInput")
    d_xtk = nc.dram_tensor("xtk", [KC, 128, S], BF, kind="ExternalInput")
    d_xtv = nc.dram_tensor("xtv", [KC, 128, S], BF, kind="ExternalInput")
    d_wq = nc.dram_tensor("wq", [H, 128, 1024], BF, kind="ExternalInput")
    d_wk = nc.dram_tensor("wk", [H, 128, 1024], BF, kind="ExternalInput")
    d_wv = nc.dram_tensor("wv", [KC, 128, 1024], BF, kind="ExternalInput")
    d_wo = nc.dram_tensor("wo", [H, 128, 1024], BF, kind="ExternalInput")
    d_cst = nc.dram_tensor("cst", [128, 128], BF, kind="ExternalInput")
    d_onec = nc.dram_tensor("onec", [1, 128], MDT, kind="ExternalInput")
    d_out = nc.dram_tensor("out", [S, 1024], F32, kind="ExternalOutput")

    with tile.TileContext(nc) as tc, ExitStack() as ctx:
        ctx.enter_context(
            nc.allow_low_precision(reason="bf16 projections / fp32r attention")
        )
        pXv = ctx.enter_context(tc.tile_pool(name="xv", bufs=8))
        pXq = ctx.enter_context(tc.tile_pool(name="xq", bufs=8))
        pXk = ctx.enter_context(tc.tile_pool(name="xk", bufs=8))
        pWv = ctx.enter_context(tc.tile_pool(name="wv", bufs=8))
        pWqk = ctx.enter_context(tc.tile_pool(name="wqk", bufs=4))
        pWo = ctx.enter_context(tc.tile_pool(name="wo", bufs=8))
        pV1 = ctx.enter_context(tc.tile_pool(name="v1", bufs=8))
        pV2 = ctx.enter_context(tc.tile_pool(name="v2", bufs=16))
        pOsb = ctx.enter_context(tc.tile_pool(name="osb", bufs=8))
        pStk = ctx.enter_context(tc.tile_pool(name="stk", bufs=8))
        pE = ctx.enter_context(tc.tile_pool(name="e", bufs=11))
        pSm = ctx.enter_context(tc.tile_pool(name="sm", bufs=6))
        pOev = ctx.enter_context(tc.tile_pool(name="oev", bufs=2))
        pC = ctx.enter_context(tc.tile_pool(name="const", bufs=2))
        pZc = ctx.enter_context(tc.tile_pool(name="zc", bufs=6))
        pTree = ctx.enter_context(tc.tile_pool(name="tree", bufs=9))

        ps_work = ctx.enter_context(tc.tile_pool(name="ps_work", bufs=4, space="PSUM"))
        ps_p12 = ctx.enter_context(tc.tile_pool(name="ps_p12", bufs=2, space="PSUM"))
        ps_sums = ctx.enter_context(tc.tile_pool(name="ps_sums", bufs=2, space="PSUM"))

        cst = pC.tile([128, 128], BF, tag="cst")
        nc.sync.dma_start(out=cst, in_=d_cst[:, :])
        ones128 = cst
        onec = pC.tile([1, 128], MDT, tag="cst", name="onec")
        nc.sync.dma_start(out=onec, in_=d_onec[:, :])

        # ---- input DMA: xtv/wv pairwise so V-proj mm kk unblocks early,
        # split across two DGE queues (sync + vector) for bandwidth ----
        xtv = []
        wv = []
        for kk in range(KC):
            t = pXv.tile([128, S], BF, tag="xv", name=f"xtv{kk}")
            nc.sync.dma_start(out=t, in_=d_xtv[kk])
            xtv.append(t)
            t = pWv.tile([128, 1024], BF, tag="wv", name=f"wv{kk}")
            nc.scalar.dma_start(out=t, in_=d_wv[kk])
            wv.append(t)
        xtk = []
        xtq = []
        for kk in range(KC):
            t = pXk.tile([128, S], BF, tag="xk", name=f"xtk{kk}")
            nc.sync.dma_start(out=t, in_=d_xtk[kk])
            xtk.append(t)
            t = pXq.tile([128, S], BF, tag="xq", name=f"xtq{kk}")
            nc.scalar.dma_start(out=t, in_=d_xtq[kk])
            xtq.append(t)
        wk_t = {}
        wq_t = {}
        wk_t[0] = pWqk.tile([128, 1024], BF, tag="wqk", name="wk0")
        nc.sync.dma_start(out=wk_t[0], in_=d_wk[0])
        wq_t[0] = pWqk.tile([128, 1024], BF, tag="wqk", name="wq0")
        nc.scalar.dma_start(out=wq_t[0], in_=d_wq[0])

        # per-head state
        qstack = {}
        kneg = {}
        kswap = {}
        v2h = {}
        v1 = []
        osb = []
        wo_t = []

        def proj_k_gen(h):
            """K projection for head h -> kneg [kr;-ki], kswap [ki;kr].
            Yields every 4 matmuls so callers can interleave PE work."""
            kst = pStk.tile([128, S], MDT, tag="stk", name=f"kst{h}")
            kng = pStk.tile([128, S], MDT, tag="stk", name=f"kng{h}")
            ksw = pStk.tile([128, S], MDT, tag="stk", name=f"ksw{h}")
            kneg[h] = kng
            kswap[h] = ksw
            for nh in range(2):
                sl = slice(nh * 512, (nh + 1) * 512)
                ps = ps_work.tile([128, 512], F32, tag="ps_work", name=f"psk{h}{nh}")
                for kk in range(KC):
                    nc.tensor.matmul(
                        ps,
                        lhsT=wk_t[h][:, kk * 128 : (kk + 1) * 128],
                        rhs=xtk[kk][:, sl],
                        start=(kk == 0),
                        stop=(kk == KC - 1),
                    )
                    if kk % 4 == 3:
                        yield
                nc.vector.tensor_copy(kst[:, sl], ps)
                nc.vector.tensor_copy(kng[0:64, sl], ps[0:64, :])
                nc.vector.tensor_scalar_mul(kng[64:128, sl], ps[64:128, :], -1.0)
            nc.sync.dma_start(out=ksw[0:64, :], in_=kst[64:128, :])
            nc.sync.dma_start(out=ksw[64:128, :], in_=kst[0:64, :])

        def proj_q_gen(h):
            qst = pStk.tile([128, S], MDT, tag="stk", name=f"qst{h}")
            qstack[h] = qst
            for nh in range(2):
                sl = slice(nh * 512, (nh + 1) * 512)
                ps = ps_work.tile([128, 512], F32, tag="ps_work", name=f"psq{h}{nh}")
                for kk in range(KC):
                    nc.tensor.matmul(
                        ps,
                        lhsT=wq_t[h][:, kk * 128 : (kk + 1) * 128],
                        rhs=xtq[kk][:, sl],
                        start=(kk == 0),
                        stop=(kk == KC - 1),
                    )
                    if kk % 4 == 3:
                        yield
                nc.vector.tensor_copy(qst[:, sl], ps)

        def emit_v2(h):
            lst = []
            base = h * 128
            for tk in range(TC):
                vt = pV2.tile([128, 128], BF, tag="v2")
                nc.gpsimd.tensor_scalar_mul(
                    vt[:, 0:64], v1[tk][:, base + 64 : base + 128], -1.0
                )
                nc.gpsimd.tensor_copy(vt[:, 64:128], v1[tk][:, base : base + 64])
                lst.append(vt)
            v2h[h] = lst

        # ---- V projection with head-0 proj blocks interleaved ----
        for t_ in range(TC):
            vt = pV1.tile([128, 1024], BF, tag="v1")
            for nh in range(2):
                ps = ps_work.tile([128, 512], F32, tag="ps_work")
                for kk in range(KC):
                    nc.tensor.matmul(
                        ps,
                        lhsT=xtv[kk][:, t_ * 128 : (t_ + 1) * 128],
                        rhs=wv[kk][:, nh * 512 : (nh + 1) * 512],
                        start=(kk == 0),
                        stop=(kk == KC - 1),
                    )
                nc.vector.tensor_copy(vt[:, nh * 512 : (nh + 1) * 512], ps)
            v1.append(vt)
            if t_ == 3:
                pk0 = proj_k_gen(0)
                next(pk0, None)
                next(pk0, None)
            elif t_ == 4:
                for _ in pk0:
                    pass
            elif t_ == 5:
                pq0 = proj_q_gen(0)
                next(pq0, None)
                next(pq0, None)
            elif t_ == 6:
                for _ in pq0:
                    pass
        emit_v2(0)

        # ---- attention per head, software pipelined ----
        pending_s1 = []
        pending_s2 = []

        def drain_s1():
            for fn in pending_s1:
                fn()
            pending_s1.clear()

        def drain_s2():
            for fn in pending_s2:
                fn()
            pending_s2.clear()

        def emit_attn(h, nh, ot, gen=None):
            nsl = slice(nh * 512, (nh + 1) * 512)
            p1 = ps_p12.tile([128, 512], F32, tag="ps_p12")
            p2 = ps_p12.tile([128, 512], F32, tag="ps_p12")
            es = {}
            tree = {0: [], 1: []}  # per comp: level-1/2 partials

            def emit_st(tk):
                ksl = slice(tk * 128, (tk + 1) * 128)
                for comp, lhsT_k in ((0, kneg[h]), (1, kswap[h])):
                    st = ps_work.tile([128, 512], F32, tag="ps_work")
                    nc.tensor.matmul(
                        st, lhsT=lhsT_k[:, ksl], rhs=qstack[h][:, nsl],
                        start=True, stop=True,
                    )
                    e = pE.tile([128, 512], BF, tag="e")
                    nc.scalar.activation(e, st, func=mybir.ActivationFunctionType.Exp)
                    es[(tk, comp)] = e

            E = {}

            def emit_av(tk):
                for comp in range(2):
                    e = es.pop((tk, comp))
                    pdst = p1 if comp == 0 else p2
                    vt = (
                        v1[tk][:, h * 128 : (h + 1) * 128]
                        if comp == 0
                        else v2h[h][tk]
                    )
                    nc.tensor.matmul(
                        pdst, lhsT=vt, rhs=e,
                        start=(tk == 0), stop=(tk == TC - 1),
                    )
                    # bf16 balanced pair-add tree on DVE replaces the
                    # per-tk ones-matmuls (partition reduce deferred to one
                    # M=1 matmul per comp in stage1)
                    tr = tree[comp]
                    tr.append(e)
                    if tk % 2 == 1:
                        t = pTree.tile([128, 512], BF, tag="tree", name="l1")
                        nc.vector.tensor_add(t, tr[-2], tr[-1])
                        tr[-2:] = [t]
                    if tk == 3 or tk == TC - 1:
                        t = pTree.tile([128, 512], BF, tag="tree", name="l2")
                        nc.vector.tensor_add(t, tr[-2], tr[-1])
                        tr[-2:] = [t]
                    if tk == TC - 1:
                        t = pTree.tile([128, 512], BF, tag="tree", name="l3")
                        nc.vector.tensor_add(t, tr[0], tr[1])
                        E[comp] = t

            emit_st(0)
            emit_st(1)
            for i in range(2, 8):
                if gen is not None:
                    next(gen, None)
                emit_st(i)
                if i == 3:
                    drain_s1()
            if gen is not None:
                for _ in gen:
                    pass
            drain_s2()
            for tk in range(TC):
                emit_av(tk)

            # Deferred normalize, stage 1: M=1 ones-matmuls give Z rows in
            # PSUM; DVE evacuates P1/P2 + Z rows, compacts Z to [128,8] via
            # DMA, one tiny reciprocal, DMA back to [1,512] rows.
            def stage1(p1=p1, p2=p2, E=E):
                s1 = pSm.tile([128, 512], F32, tag="sm", name="s1")
                nc.vector.tensor_copy(s1, p1)
                s2 = pSm.tile([128, 512], F32, tag="sm", name="s2")
                nc.vector.tensor_copy(s2, p2)
                sums_r = ps_sums.tile([1, 512], F32, tag="ps_sums", name="zr")
                nc.tensor.matmul(
                    sums_r, lhsT=ones128[:, 0:1], rhs=E[0], start=True, stop=True
                )
                sums_i = ps_sums.tile([1, 512], F32, tag="ps_sums", name="zi")
                nc.tensor.matmul(
                    sums_i, lhsT=ones128[:, 0:1], rhs=E[1], start=True, stop=True
                )
                zrow_r = pZc.tile([1, 512], F32, tag="zc", name="zrowr")
                nc.vector.tensor_copy(zrow_r, sums_r)
                zrow_i = pZc.tile([1, 512], F32, tag="zc", name="zrowi")
                nc.vector.tensor_copy(zrow_i, sums_i)
                zc = pZc.tile([128, 8], F32, tag="zc")
                nc.sync.dma_start(out=zc[:, 0:4], in_=zrow_r)
                nc.sync.dma_start(out=zc[:, 4:8], in_=zrow_i)
                rc = pZc.tile([128, 8], MDT, tag="zc")
                nc.vector.reciprocal(rc, zc)
                row_r = pZc.tile([1, 512], MDT, tag="zc", name="rowr")
                nc.sync.dma_start(out=row_r, in_=rc[:, 0:4])
                row_i = pZc.tile([1, 512], MDT, tag="zc", name="rowi")
                nc.sync.dma_start(out=row_i, in_=rc[:, 4:8])
                st2 = {"s1": s1, "s2": s2, "row_r": row_r, "row_i": row_i}
                return st2

            box = {}

            def stage1_wrap(box=box):
                box.update(stage1())

            # stage 2: K=1 ones-matmuls broadcast the reciprocals across
            # partitions; DVE muls + GpSimd add assemble ot.
            def stage2(box=box, ot=ot, nsl=nsl):
                pb_r = ps_work.tile([128, 512], F32, tag="ps_work", name="pbr")
                nc.tensor.matmul(
                    pb_r, lhsT=onec, rhs=box["row_r"], start=True, stop=True
                )
                pb_i = ps_work.tile([128, 512], F32, tag="ps_work", name="pbi")
                nc.tensor.matmul(
                    pb_i, lhsT=onec, rhs=box["row_i"], start=True, stop=True
                )
                t1 = pSm.tile([128, 512], F32, tag="sm", name="t1")
                nc.vector.tensor_mul(t1, box["s1"], pb_r)
                t2 = pSm.tile([128, 512], F32, tag="sm", name="t2")
                nc.vector.tensor_mul(t2, box["s2"], pb_i)
                nc.gpsimd.tensor_add(ot[:, nsl], t1, t2)

            pending_s1.append(stage1_wrap)
            pending_s2.append(stage2)

        for h in range(H):
            ot = pOsb.tile([128, S], BF, tag="osb")
            if h + 1 < H:
                wk_t[h + 1] = pWqk.tile([128, 1024], BF, tag="wqk", name=f"wk{h+1}")
                nc.sync.dma_start(out=wk_t[h + 1], in_=d_wk[h + 1])
                wq_t[h + 1] = pWqk.tile([128, 1024], BF, tag="wqk", name=f"wq{h+1}")
                nc.sync.dma_start(out=wq_t[h + 1], in_=d_wq[h + 1])
                emit_v2(h + 1)
            if h == 5:
                # wo DMA: late, overlaps heads 6-7
                for hh in range(H):
                    t = pWo.tile([128, 1024], BF, tag="wo")
                    nc.sync.dma_start(out=t, in_=d_wo[hh])
                    wo_t.append(t)
            emit_attn(h, 0, ot, proj_k_gen(h + 1) if h + 1 < H else None)
            emit_attn(h, 1, ot, proj_q_gen(h + 1) if h + 1 < H else None)
            osb.append(ot)

        # ---- output projection ----
        # Phase A: first 8 chunks accumulate heads 0..6 while the last
        # head's deferred normalize chain completes; drains interleave so
        # the PE never idles waiting on it. Chunk PSUMs borrow from all
        # three pools (banks are fungible).
        chunks = [(t_, nh) for t_ in range(TC) for nh in range(2)]
        opart = {}

        def o_mm(ps, t_, nh, h, start):
            nc.tensor.matmul(
                ps,
                lhsT=osb[h][:, t_ * 128 : (t_ + 1) * 128],
                rhs=wo_t[h][:, nh * 512 : (nh + 1) * 512],
                start=start,
                stop=(h == H - 1),
            )

        pool_for = [(ps_work, "ps_work"), (ps_work, "ps_work"),
                    (ps_p12, "ps_p12"), (ps_p12, "ps_p12"),
                    (ps_sums, "ps_sums"), (ps_sums, "ps_sums"),
                    (ps_work, "ps_work"), (ps_work, "ps_work")]
        for ci in range(8):
            t_, nh = chunks[ci]
            pool, tg = pool_for[ci]
            ps = pool.tile([128, 512], F32, tag=tg, name=f"op{ci}")
            for h in range(H - 1):
                o_mm(ps, t_, nh, h, start=(h == 0))
            opart[ci] = ps
            if ci == 1:
                drain_s1()
            elif ci == 3:
                drain_s2()
        # Phase B: finish first 8 chunks with head 7, then do the rest
        for ci in range(8):
            t_, nh = chunks[ci]
            ps = opart.pop(ci)
            o_mm(ps, t_, nh, H - 1, start=False)
            oev = pOev.tile([128, 512], F32, tag="oev")
            nc.scalar.copy(oev, ps)
            nc.sync.dma_start(
                out=d_out[t_ * 128 : (t_ + 1) * 128, nh * 512 : (nh + 1) * 512],
                in_=oev,
            )
        for ci in range(8, 16):
            t_, nh = chunks[ci]
            ps = ps_work.tile([128, 512], F32, tag="ps_work")
            for h in range(H):
                o_mm(ps, t_, nh, h, start=(h == 0))
            oev = pOev.tile([128, 512], F32, tag="oev")
            nc.scalar.copy(oev, ps)
            nc.sync.dma_start(
                out=d_out[t_ * 128 : (t_ + 1) * 128, nh * 512 : (nh + 1) * 512],
                in_=oev,
            )

    _split_waits(nc)
    return nc


_NC_CACHE = {}


def kernel(
    queries,
    keys,
    values,
    wq_r,
    wq_i,
    wk_r,
    wk_i,
    wv_r,
    wv_i,
    wo_r,
    wo_i,
    _trace=False,
):
    global LAST_EXEC_NS
    _install_axon_profile_shim()
    _install_tile_drain_patch()
    from concourse.bass_utils import run_bass_kernel_spmd

    scale = 1.0 / np.sqrt(DH)
    WQ = _bf16(_head_tiles(_build_wqk(np.asarray(wq_r), np.asarray(wq_i), scale)))
    WK = _bf16(_head_tiles(_build_wqk(np.asarray(wk_r), np.asarray(wk_i), 1.0)))
    WV = _bf16(_kchunk_tiles(_build_wqk(np.asarray(wv_r), np.asarray(wv_i), 1.0)))
    WO = _bf16(_kchunk_tiles(_build_wo(np.asarray(wo_r), np.asarray(wo_i))))
    CST = np.ones((128, 128), ml_dtypes.bfloat16)
    ONEC = np.ones((1, 128), np.float32)

    queries = np.asarray(queries)
    keys = np.asarray(keys)
    values = np.asarray(values)

    in_maps = []
    for b in range(NCORES):
        in_maps.append(
            {
                "xtq": _bf16(_xt(queries[b]).reshape(KC, 128, S)),
                "xtk": _bf16(_xt(keys[b]).reshape(KC, 128, S)),
                "xtv": _bf16(_xt(values[b]).reshape(KC, 128, S)),
                "wq": WQ,
                "wk": WK,
                "wv": WV,
                "wo": WO,
                "cst": CST,
                "onec": ONEC,
            }
        )

    if "nc" not in _NC_CACHE:
        _NC_CACHE["nc"] = _build_nc()
    nc = _NC_CACHE["nc"]

    res = run_bass_kernel_spmd(nc, in_maps, list(range(NCORES)), trace=_trace)
    LAST_EXEC_NS = res.exec_time_ns

    out = np.empty((B, S, D, 2), np.float32)
    for b in range(NCORES):
        out[b] = res.results[b]["out"].reshape(S, D, 2)
    return out
